# revision 31
# baseline (speedup 1.0000x reference)
"""Trainium2 Bass kernel for nn_GCEdecoder (sparse_attention).

Reference computation (B=128, T=512, D=400, V=1024, A=128):
  vals = C_vals[:,0,:]                               # [V, D]
  S[b,v,t]  = sum_d H[b,t,d] * vals[v,d]             # scores
  P         = softmax over t (masked t < len_b)
  y_utts[b,v] = sum_d (sum_t P[b,v,t] H[b,t,d]) * W[d] + b0
  s2[b,a]   = sum_d C_acts[b,a,d] * c_utt[b,d]
  p2        = softmax_a(s2);  q[b,d] = sum_a p2 C_acts[b,a,d]
  y_acts[b,v] = sum_d q[b,d] vals[v,d]

Key restructure: y_utts[b,v] = (sum_t E[t,v]*hwm[b,t]) / (sum_t E[t,v]*m[b,t])
with E = exp(S - U_b) and hwm = (H@W + b0)*mask, m = mask.  This removes the
second big einsum (q_utts) entirely; masking and bias fold into a [T,33]
scoring matrix (numerator row lands on psum partition 0, denominator on the
32-aligned partition the DVE can address).  A per-batch shift U_b (host
estimate from sampled true scores) replaces the per-row max — softmax is
shift-invariant, so the result is exact after the num/den division, and exp
stays comfortably inside fp32 range.

Sharding: data-parallel over B across 8 cores (16 batches/core); vals and the
scoring matrix are replicated.  All heavy matmuls run in float32r (1 cycle/row
on the PE at N>=256, ~1.5e-4 relative accuracy).  Per-core modeled duration:
~158 us (TimelineSim), ~96% PE occupancy.
"""

import os
import time

import numpy as np

import concourse.bacc as bacc
import concourse.mybir as mybir
import concourse.tile as tile
from concourse.bass_utils import run_bass_kernel_spmd

B, T, D, V, A = 128, 512, 400, 1024, 128
NCORES = 8
BPC = B // NCORES  # batches per core
F32 = mybir.dt.float32
F32R = mybir.dt.float32r
EXP = mybir.ActivationFunctionType.Exp
SHIFT = 65.0  # legacy constant used by debug harnesses only

_cache = {}

HT_BUFS = int(os.environ.get("HT_BUFS", "2"))
E_BUFS = int(os.environ.get("E_BUFS", "4"))
PSS_BUFS = int(os.environ.get("PSS_BUFS", "2"))
PSY_BUFS = int(os.environ.get("PSY_BUFS", "3"))
SWIDE = int(os.environ.get("SWIDE", "1024"))


def build_program():
    nc = bacc.Bacc("TRN2", target_bir_lowering=False, debug=False)

    # Per-core inputs (host pre-swizzled; see kernel() below).
    ht = nc.dram_tensor("ht", (BPC, 4, 128, T), F32R, kind="ExternalInput")
    smt = nc.dram_tensor("smt", (128, BPC, 4, 33), F32R, kind="ExternalInput")
    ca = nc.dram_tensor("ca", (BPC, A, 512), F32, kind="ExternalInput")
    cu = nc.dram_tensor("cu", (BPC, D), F32, kind="ExternalInput")
    vt = nc.dram_tensor("vt", (128, 4, V), F32R, kind="ExternalInput")
    shf = nc.dram_tensor("shf", (128, 2 * BPC), F32, kind="ExternalInput")
    yu = nc.dram_tensor("yu", (1, BPC, 2, 512), F32, kind="ExternalOutput")
    ya = nc.dram_tensor("ya", (BPC, V), F32, kind="ExternalOutput")

    with tile.TileContext(nc) as tc:
        with (
            tc.tile_pool(name="const", bufs=1) as cpool,
            tc.tile_pool(name="work", bufs=HT_BUFS) as wpool,
            tc.tile_pool(name="etile", bufs=E_BUFS) as epool,
            tc.tile_pool(name="psS", bufs=PSS_BUFS, space="PSUM") as psS,
            tc.tile_pool(name="psY", bufs=PSY_BUFS, space="PSUM") as psY,
            tc.tile_pool(name="psQ", bufs=1, space="PSUM") as psQ,
        ):
            # ---- constants / persistent tiles (DMAs emitted at b==0) ----
            vt_sb = cpool.tile([128, 4, V], F32R)
            sm_sb = cpool.tile([128, BPC, 4, 33], F32R)

            bias_sb = cpool.tile([128, 2 * BPC], F32)
            nc.sync.dma_start(bias_sb[:], shf[:])
            onecol_sb = cpool.tile([128, 1], F32)
            nc.vector.memset(onecol_sb[:], 1.0)
            warm_sb = cpool.tile([128, 512], F32)
            nc.vector.memset(warm_sb[:], 0.0)
            warm_ps = psY.tile([1, 512], F32, tag="y")
            for _ in range(4):
                nc.tensor.matmul(
                    warm_ps[:], onecol_sb[:, :1], warm_sb[:], start=True, stop=True
                )

            # y_utts numerator staging / final values: [1, b, vc, 512]
            nd_sb = cpool.tile([1, BPC, 2, 512], F32)
            # q^T accumulator across batches: [d-part, dchunk, b]
            qt_sb = cpool.tile([128, 4, BPC], F32R)
            d2_sb = cpool.tile([128, BPC], F32)
            nc.vector.memset(d2_sb[:], 0.0)
            yacts_sb = cpool.tile([BPC, V], F32)

            pend = []
            y_tiles_by_b = {}

            def _flush_y(item):
                e_sb, bb, vc, jth = item
                y_ps = y_tiles_by_b[(bb, vc)]
                for half in range(SWIDE // 512):
                    jt = (SWIDE // 512) * jth + half
                    nc.tensor.matmul(
                        y_ps[:],
                        sm_sb[:, bb, jt, :],
                        e_sb[:, 512 * half : 512 * (half + 1)],
                        start=(jt == 0),
                        stop=(jt == 3),
                    )
                if jth == (4 // (SWIDE // 512)) - 1:
                    nc.vector.tensor_copy(nd_sb[:, bb, vc, :], y_ps[0:1, :])
                    nc.vector.reciprocal(y_ps[32:33, :], y_ps[32:33, :])
                    nc.vector.tensor_tensor(
                        nd_sb[:, bb, vc, :],
                        nd_sb[:, bb, vc, :],
                        y_ps[32:33, :],
                        mybir.AluOpType.mult,
                    )

            for b in range(BPC):
                # ---- load this batch ----
                ht_sb = wpool.tile([128, 4, T], F32R, tag="ht")
                for j in range(4):
                    nc.sync.dma_start(ht_sb[:, j, :], ht[b, j])
                if b == 0:
                    for jd in range(4):
                        nc.sync.dma_start(vt_sb[:, jd, 0:512], vt[:, jd, 0:512])
                    for jd in range(4):
                        nc.sync.dma_start(vt_sb[:, jd, 512:1024], vt[:, jd, 512:1024])
                    nc.scalar.dma_start(sm_sb[:], smt[:])
                ca_sb = wpool.tile([128, 512], F32, tag="ca")
                nc.scalar.dma_start(ca_sb[:], ca[b])

                # ---- y_acts front half: s2 -> p2 -> q^T chunks ----
                cb_sb = wpool.tile([128, D], F32, tag="cb")
                nc.scalar.dma_start(cb_sb[:], cu[b : b + 1, :].to_broadcast((128, D)))
                scr_sb = epool.tile([128, D], F32, tag="scr")
                s2_sb = epool.tile([128, 1], F32, tag="s2")
                nc.vector.tensor_tensor(
                    scr_sb[:], ca_sb[:, 0:D], cb_sb[:], mybir.AluOpType.mult
                )
                nc.vector.tensor_reduce(
                    s2_sb[:], scr_sb[:], mybir.AxisListType.X, mybir.AluOpType.add
                )
                p2_sb = epool.tile([128, 1], F32, tag="p2")
                nc.scalar.activation(p2_sb[:], s2_sb[:], EXP, bias=bias_sb[:, BPC + b : BPC + b + 1])

                qt_ps = psQ.tile([128, 5], F32, tag="qt")
                for j in range(4):
                    nc.tensor.matmul(
                        qt_ps[:, j : j + 1],
                        ca_sb[:, 128 * j : 128 * (j + 1)],
                        p2_sb[:],
                        start=True,
                        stop=True,
                    )
                nc.tensor.matmul(
                    qt_ps[0:1, 4:5], p2_sb[:], onecol_sb[:], start=True, stop=True
                )
                nc.scalar.copy(qt_sb[:, :, b], qt_ps[:, 0:4])
                nc.scalar.copy(d2_sb[0:1, b : b + 1], qt_ps[0:1, 4:5])

                # ---- scores + exp + num/den (Y matmuls lag one tile) ----
                nhalf = SWIDE // 512
                for vc in range(2):
                    y_tiles_by_b[(b, vc)] = psY.tile([33, 512], F32, tag="y", name=f"y_ps_{b}_{vc}")
                for step in range(2 * (4 // nhalf)):
                    vc, jth = divmod(step, 4 // nhalf)
                    s_ps = psS.tile([128, SWIDE], F32, tag="s")
                    for half in range(nhalf):
                        jt = nhalf * jth + half
                        for jd in range(4):
                            nc.tensor.matmul(
                                s_ps[:, 512 * half : 512 * (half + 1)],
                                ht_sb[:, jd, 128 * jt : 128 * (jt + 1)],
                                vt_sb[:, jd, 512 * vc : 512 * (vc + 1)],
                                start=(jd == 0),
                                stop=(jd == 3),
                            )
                    e_sb = epool.tile([128, SWIDE], F32R, tag="e")
                    nc.scalar.activation(e_sb[:], s_ps[:], EXP, bias=bias_sb[:, b : b + 1])
                    pend.append((e_sb, b, vc, jth))
                    if len(pend) > 1:
                        _flush_y(pend.pop(0))
                # defer tail tile into next batch; drain handled after loop

            while pend:
                _flush_y(pend.pop(0))

            # ---- epilogue: y_acts = (qT.T @ valsT) / d2 ----
            d2t_ps = psQ.tile([BPC, 1], F32, tag="qt")
            nc.tensor.matmul(d2t_ps[:], d2_sb[:], onecol_sb[:], start=True, stop=True)
            d2t_sb = epool.tile([BPC, 1], F32, tag="d2t")
            nc.vector.reciprocal(d2t_sb[:], d2t_ps[:])
            for vc in range(2):
                ya_ps = psY.tile([BPC, 512], F32, tag="y")
                for j in range(4):
                    nc.tensor.matmul(
                        ya_ps[:],
                        qt_sb[:, j, :],
                        vt_sb[:, j, 512 * vc : 512 * (vc + 1)],
                        start=(j == 0),
                        stop=(j == 3),
                    )
                nc.vector.tensor_scalar(
                    yacts_sb[:, 512 * vc : 512 * (vc + 1)],
                    ya_ps[:],
                    d2t_sb[:],
                    None,
                    mybir.AluOpType.mult,
                )

            nc.sync.dma_start(yu[:], nd_sb[0:1, :, :, :])
            nc.sync.dma_start(ya[:], yacts_sb[:])

    nc.compile()
    return nc


def _prep_inputs(H_utt, c_utt, C_acts, C_vals, W_score, b_score, utterance_len):
    """Host-side reshaping/swizzling into the kernel's per-core layouts."""
    H_utt = np.ascontiguousarray(H_utt, dtype=np.float32)
    c_utt = np.asarray(c_utt, dtype=np.float32)
    C_acts = np.asarray(C_acts, dtype=np.float32)
    ca_pad = np.zeros((B, A, 512), np.float32)
    ca_pad[:, :, :D] = C_acts
    vals = np.asarray(C_vals, dtype=np.float32)[:, 0, :]  # [V, D]
    W = np.asarray(W_score, dtype=np.float32)[0]  # [D]
    b0 = np.float32(np.asarray(b_score, dtype=np.float32).reshape(-1)[0])
    lens = np.asarray(utterance_len).astype(np.int64)

    # H^T padded to 512 rows: [B, 4, 128, T]
    htp = np.zeros((B, 512, T), np.float32)
    htp[:, :D, :] = H_utt.transpose(0, 2, 1)
    ht_all = htp.reshape(B, 4, 128, T)

    # valsT swizzled: vt[p, j, v] = vals[v, 128j+p], zero-padded past D
    vtp = np.zeros((512, V), np.float32)
    vtp[:D] = vals.T
    vt_host = np.ascontiguousarray(vtp.reshape(4, 128, V).transpose(1, 0, 2))

    # scoring matrix [B, T, 2] = (hw*mask, mask), laid out [128, B, 4, 2]
    hw = H_utt.reshape(B * T, D) @ W
    hw = hw.reshape(B, T) + b0
    mask = (np.arange(T)[None, :] < lens[:, None]).astype(np.float32)
    sm = np.zeros((B, T, 33), np.float32)
    sm[:, :, 0] = hw * mask
    sm[:, :, 32] = mask
    # t = 128*j + p  ->  [B, 4(j), 128(p), 33] -> [128, B, 4, 33]
    sm_host = np.ascontiguousarray(
        sm.reshape(B, 4, 128, 33).transpose(2, 0, 1, 3)
    )

    # Per-batch exp shifts (exact after normalization: num and den share the
    # exp(-shift) factor).  y_utts: 0.85x a strided-sample max of the true
    # scores — keeps exp within fp32 for input scales up to ~2.5x nominal
    # while never flushing the denominator.  y_acts: exact row max (cheap).
    s_samp = np.einsum(
        "btd,vd->btv", H_utt[:, ::8, :].astype(np.float64), vals[::8].astype(np.float64)
    )
    shift_u = np.maximum(0.85 * s_samp.max(axis=(1, 2)), 1.0)  # [B]
    s2_full = np.einsum("bad,bd->ba", C_acts.astype(np.float64), c_utt.astype(np.float64))
    shift_a = s2_full.max(axis=1)  # [B]
    shifts = np.stack([shift_u, shift_a], axis=0).astype(np.float32)  # [2, B]

    in_maps = []
    for c in range(NCORES):
        s = slice(c * BPC, (c + 1) * BPC)
        in_maps.append(
            {
                "ht": np.ascontiguousarray(ht_all[s]),
                "smt": np.ascontiguousarray(sm_host[:, s]),
                "ca": np.ascontiguousarray(ca_pad[s]),
                "cu": np.ascontiguousarray(c_utt[s]),
                "vt": vt_host,
                "shf": np.ascontiguousarray(
                    np.broadcast_to(
                        -np.concatenate([shifts[0, s], shifts[1, s]])[None, :],
                        (128, 2 * BPC),
                    )
                ),
            }
        )
    return in_maps


def _run_with_retry(nc, in_maps, attempts=4):
    """First execution of a freshly compiled NEFF occasionally dies with
    NRT_EXEC_UNIT_UNRECOVERABLE on this deployment; the terminal recovers and
    an immediate retry succeeds.  Retry a few times before giving up."""
    last = None
    for i in range(attempts):
        try:
            return run_bass_kernel_spmd(nc, in_maps, core_ids=list(range(NCORES)))
        except Exception as e:  # noqa: BLE001 - any runtime/transport error
            last = e
            time.sleep(2.0 * (i + 1))
    raise last


def kernel(H_utt, c_utt, C_acts, C_vals, W_score, b_score, utterance_len, **_):
    if "nc" not in _cache:
        _cache["nc"] = build_program()
    nc = _cache["nc"]

    in_maps = _prep_inputs(
        H_utt, c_utt, C_acts, C_vals, W_score, b_score, utterance_len
    )
    res = _run_with_retry(nc, in_maps)

    y_utts = np.concatenate(
        [r["yu"].reshape(BPC, V) for r in res.results], axis=0
    ).astype(np.float32)
    y_acts = np.concatenate([r["ya"] for r in res.results], axis=0).astype(np.float32)
    return (y_utts, y_acts)


def kernel_traced(trace=True, **inputs):
    """Like kernel() but returns (outputs, BassKernelResults) with profiling."""
    if "nc" not in _cache:
        _cache["nc"] = build_program()
    nc = _cache["nc"]
    in_maps = _prep_inputs(**{k: inputs[k] for k in (
        "H_utt", "c_utt", "C_acts", "C_vals", "W_score", "b_score", "utterance_len")})
    res = run_bass_kernel_spmd(
        nc, in_maps, core_ids=list(range(NCORES)), trace=trace
    )
    y_utts = np.concatenate(
        [r["yu"].reshape(BPC, V) for r in res.results], axis=0
    ).astype(np.float32)
    y_acts = np.concatenate([r["ya"] for r in res.results], axis=0).astype(np.float32)
    return (y_utts, y_acts), res


if __name__ == "__main__":
    rng = np.random.default_rng(0)
    inputs = {
        "H_utt": rng.standard_normal((B, T, D), dtype=np.float32),
        "c_utt": rng.standard_normal((B, D), dtype=np.float32),
        "C_acts": rng.standard_normal((B, A, D), dtype=np.float32),
        "C_vals": rng.standard_normal((V, 1, D), dtype=np.float32),
        "W_score": rng.standard_normal((1, D), dtype=np.float32) / np.sqrt(D),
        "b_score": np.zeros((1,), np.float32),
        "utterance_len": rng.integers(T // 2, T + 1, size=(B,)).astype(np.int64),
    }
    y_utts, y_acts = kernel(**inputs)
    print("y_utts", y_utts.shape, "y_acts", y_acts.shape)

